# revision 1
# baseline (speedup 1.0000x reference)
import sys
if '/opt/trn_rl_repo' not in sys.path:
    sys.path.insert(0, '/opt/trn_rl_repo')
import numpy as np
import ml_dtypes

import concourse.bass as bass
import concourse.bacc as bacc
import concourse.tile as tile
from concourse import mybir
from concourse.bass_utils import run_bass_kernel_spmd
from concourse.masks import make_identity

F32 = mybir.dt.float32
BF = mybir.dt.bfloat16
AF = mybir.ActivationFunctionType
MUL = mybir.AluOpType.mult
ADD = mybir.AluOpType.add
SUB = mybir.AluOpType.subtract
P = 128
D, H, DK, DV, NL = 768, 8, 64, 64, 2
B, LC, LQ, LK = 8, 512, 160, 512
DC = D // P      # 6 chunks of the 768 dim
CC = LC // P     # 4 chunks of the 512 token dim
QCH = [(0, 128), (128, 32)]   # (offset, size) chunks of LQ=160
SCALE = 0.125    # log_512(512)/sqrt(64)
EPS = 1e-6
NPBF = ml_dtypes.bfloat16

USE_SPART = True
USE_APPROX_RECIP = True
USE_GPS_BCAST = True
_CACHE = {}


def _build():
    nc = bacc.Bacc()
    dt = {}

    def din(name, shape, dtype=BF):
        dt[name] = nc.dram_tensor(name, list(shape), dtype, kind="ExternalInput")
        return dt[name]

    # all big tensors pre-tiled on host to [128, chunks*width] (partition-contiguous)
    din('S_nat', (P, CC * D)); din('S_T', (P, DC * LC))
    din('Q_nat', (P, 2 * D)); din('Q_T', (P, DC * LQ))
    din('E_nat', (P, 2 * D)); din('E_T', (P, DC * LQ))
    din('KE_T', (P, DC * LK))
    din('vecs', (P, DC * 4), F32)    # cols: w4C, w4Q, w4mlu, cqa_b
    din('cqa_WT', (P, 4 * DC * D))
    for l in range(NL):
        din(f'sWq{l}', (P, DC * H * DK)); din(f'sWk{l}', (P, DC * H * DK))
        din(f'sWv{l}', (P, DC * H * DV)); din(f'sWfc{l}', (P, 4 * D))
        din(f'cWq{l}', (P, DC * H * DK)); din(f'cWk{l}', (P, 2 * DC * H * DK))
        din(f'cWv{l}', (P, 2 * DC * H * DV)); din(f'cWfc{l}', (P, 4 * D))
        din(f'ln{l}', (P, DC * 4), F32)   # cols: n1g, n1b, n2g, n2b
    out_t = nc.dram_tensor('out_t', [3 * D, LC], BF, kind="ExternalOutput")

    with tile.TileContext(nc) as tc:
        _emit(nc, tc, dt, out_t)
    nc.compile()
    return nc


def _emit(nc, tc, dt, out_t):
    from contextlib import ExitStack
    ctx = ExitStack()
    const = ctx.enter_context(tc.tile_pool(name="const", bufs=1))
    persist = ctx.enter_context(tc.tile_pool(name="persist", bufs=1))

    ident = const.tile([P, P], BF)
    make_identity(nc, ident)
    ones_row = const.tile([1, P], BF)
    nc.gpsimd.memset(ones_row, 1.0)
    ones_col = const.tile([P, 1], BF)
    nc.gpsimd.memset(ones_col, 1.0)
    eps_t = const.tile([1, 1], F32)
    nc.gpsimd.memset(eps_t, EPS)

    # ---- weight pool for layer 0 (tiles alloc'd early for LIFO order;
    # DMAs emitted after the input DMAs) ----
    def alloc_wl(l, wl, names):
        w = {}
        for nm in names:
            if nm in ('sWfc', 'cWfc'):
                w[nm] = wl.tile([P, 4, D], BF, name=f"{nm}{l}")
            elif nm == 'sWv':
                w[nm] = wl.tile([P, DC, H * DV], BF, name=f"{nm}{l}")
            else:
                w[nm] = wl.tile([P, DC, H * DK], BF, name=f"{nm}{l}")
        return w

    def dma_wl(l, w):
        for nm in w:
            nc.sync.dma_start(out=w[nm], in_=dt[f'{nm}{l}'][:, :])

    wl0pool = ctx.enter_context(tc.tile_pool(name="wl0", bufs=1))
    wls = [alloc_wl(0, wl0pool, ('sWq', 'sWk', 'sWv', 'sWfc')), None]
    ps = ctx.enter_context(tc.tile_pool(name="ps", bufs=1, space="PSUM"))

    # ---- input DMAs (batched; earliest-needed first) ----
    s2q = tc.alloc_tile_pool(name="s2q", bufs=1)
    cqaw = tc.alloc_tile_pool(name="cqaw", bufs=1)

    vecs = const.tile([P, DC, 4], F32)
    nc.sync.dma_start(out=vecs, in_=dt['vecs'][:, :])
    vecs_bf = const.tile([P, DC, 4], BF)
    nc.vector.tensor_copy(vecs_bf, vecs)

    ST3 = s2q.tile([P, DC, LC], BF, name="ST3")
    nc.sync.dma_start(out=ST3[:, 0:3, :], in_=dt['S_T'][:, 0:3 * LC])
    nc.sync.dma_start(out=ST3[:, 3:6, :], in_=dt['S_T'][:, 3 * LC:])
    S_T = [ST3[:, d, :] for d in range(DC)]

    qe_in = {}
    for tag, QN, QT in (('q', dt['Q_nat'], dt['Q_T']), ('e', dt['E_nat'], dt['E_T'])):
        qt3 = s2q.tile([P, DC, LQ], BF, name=f"Qt3{tag}")
        nc.sync.dma_start(out=qt3[:, 0:3, :], in_=QT[:, 0:3 * LQ])
        nc.sync.dma_start(out=qt3[:, 3:6, :], in_=QT[:, 3 * LQ:])
        qn3 = s2q.tile([P, 2, D], BF, name=f"Qn3{tag}")
        nc.sync.dma_start(out=qn3, in_=QN[:, :])
        Qn = [qn3[:, 0, :], qn3[:, 1, :]]
        qe_in[tag] = ([qt3[:, d, :] for d in range(DC)], Qn)

    SN3 = s2q.tile([P, CC, D], BF, name="SN3")
    nc.sync.dma_start(out=SN3, in_=dt['S_nat'][:, :])
    S_nat = [SN3[:, c, :] for c in range(CC)]

    CQ3 = cqaw.tile([P, 4 * DC, D], BF, name="CQ3")
    nc.sync.dma_start(out=CQ3, in_=dt['cqa_WT'][:, :])
    cqa_WT = [CQ3[:, k, :] for k in range(4 * DC)]

    KE3 = persist.tile([P, DC, LK], BF, name="KE3")
    nc.sync.dma_start(out=KE3, in_=dt['KE_T'][:, :])
    ke_T = [KE3[:, d, :] for d in range(DC)]

    lnv = []
    for l in range(NL):
        t = const.tile([P, DC, 4], F32, name=f"lnv{l}")
        nc.sync.dma_start(out=t, in_=dt[f'ln{l}'][:, :])
        lnv.append(t)

    dma_wl(0, wls[0])

    # ---- S-side shared prep ----
    cm3 = s2q.tile([P, DC, LC], BF, name="cm3")
    cm_T = [cm3[:, d, :] for d in range(DC)]
    for d in range(DC):
        nc.vector.tensor_scalar_mul(cm_T[d], S_T[d], vecs[:, d, 2:3])
    ps0 = ps.tile([1, LC], F32, tag="b", bufs=4)
    for d in range(DC):
        nc.tensor.matmul(ps0, vecs_bf[:, d, 0:1], S_T[d], start=(d == 0), stop=(d == DC - 1))
    s0_row = s2q.tile([1, LC], BF)
    nc.vector.tensor_copy(s0_row, ps0)
    # att held in SBUF (bf16) for phase 2 kv
    att_T = [persist.tile([P, LC], BF, name=f"attT{i}") for i in range(2 * DC)]

    # ---- s2q in stages; q/e interleaved ----
    pools = {}
    st = {}

    def stageA(tag):
        po = tc.alloc_tile_pool(name=f"s2qt_{tag}", bufs=1)
        pools[tag] = po
        Qt, Qn = qe_in[tag]
        s1 = []
        for qi, (qo, qs) in enumerate(QCH):
            pq = ps.tile([P, 1], F32, tag="b", bufs=4)
            for d in range(DC):
                nc.tensor.matmul(pq[:qs], Qt[d][:, qo:qo + qs], vecs_bf[:, d, 1:2],
                                 start=(d == 0), stop=(d == DC - 1))
            t = po.tile([P, 1], F32, name=f"s1{tag}{qi}")
            nc.vector.tensor_copy(t[:qs], pq[:qs])
            s1.append(t)
        e_t, etn = [], []
        for qi, (qo, qs) in enumerate(QCH):
            psc_t = ps.tile([P, LC], F32, tag="a", bufs=4)
            for d in range(DC):
                nc.tensor.matmul(psc_t[:qs], Qt[d][:, qo:qo + qs], cm_T[d],
                                 start=(d == 0), stop=False)
            nc.tensor.matmul(psc_t[:qs], ones_row[:1, :qs], s0_row,
                             start=False, stop=True)
            et = po.tile([P, LC], BF, name=f"et{tag}{qi}")
            stt = po.tile([P, 1], F32, name=f"st{tag}{qi}")
            nc.scalar.activation(et[:qs], psc_t[:qs], AF.Exp, bias=s1[qi][:qs],
                                 scale=1.0, accum_out=stt[:qs])
            rt = po.tile([P, 1], F32, name=f"rt{tag}{qi}")
            nc.vector.reciprocal_approx_fast(out=rt[:qs], in_=stt[:qs])
            en = po.tile([P, LC], BF, name=f"etn{tag}{qi}")
            nc.vector.tensor_scalar_mul(en[:qs], et[:qs], rt[:qs])
            e_t.append(et); etn.append(en)
        psr = ps.tile([1, LC], F32, tag="b", bufs=4)
        for qi, (qo, qs) in enumerate(QCH):
            nc.tensor.matmul(psr, ones_col[:qs, :1], e_t[qi][:qs],
                             start=(qi == 0), stop=(qi == 1))
        rc_row = po.tile([1, LC], F32, name=f"rc{tag}")
        nc.scalar.copy(rc_row, psr)
        nc.vector.reciprocal_approx_fast(out=rc_row, in_=rc_row)
        st[tag] = dict(e_t=e_t, etn=etn, rc_row=rc_row)

    def stageB(tag):
        po = pools[tag]
        s = st[tag]
        pbs = po.tile([P, LC], F32, name=f"pbs{tag}")
        nc.gpsimd.partition_broadcast(pbs, s['rc_row'])
        P_T = []
        for qi, (qo, qs) in enumerate(QCH):
            pt = po.tile([P, LC], BF, name=f"PT{tag}{qi}")
            nc.vector.tensor_tensor(pt[:qs], s['e_t'][qi][:qs], pbs[:qs], op=MUL)
            P_T.append(pt)
        etn_T = [po.tile([P, LQ], BF, name=f"etnT{tag}{c}") for c in range(CC)]
        for c in range(CC):
            for qi, (qo, qs) in enumerate(QCH):
                pt = ps.tile([P, P], BF, tag="b", bufs=4)
                nc.tensor.transpose(pt[:, :qs], s['etn'][qi][:qs, c * P:(c + 1) * P],
                                    ident[:qs, :qs])
                nc.vector.tensor_copy(etn_T[c][:, qo:qo + qs], pt[:, :qs])
        tmp = []
        for qi, (qo, qs) in enumerate(QCH):
            t = po.tile([P, D], BF, name=f"tmp{tag}{qi}")
            for n in range(2):
                pm = ps.tile([P, 384], F32, tag="a", bufs=4)
                for c in range(CC):
                    nc.tensor.matmul(pm[:qs], etn_T[c][:, qo:qo + qs],
                                     S_nat[c][:, n * 384:(n + 1) * 384],
                                     start=(c == 0), stop=(c == CC - 1))
                nc.vector.tensor_copy(t[:qs, n * 384:(n + 1) * 384], pm[:qs])
            tmp.append(t)
        s['P_T'] = P_T; s['tmp'] = tmp

    def stageC(tag, row0):
        po = pools[tag]
        s = st[tag]
        Qt, Qn = qe_in[tag]
        P_T, tmp = s['P_T'], s['tmp']
        c2q_T = [po.tile([P, LC], BF, name=f"c2qT{tag}{d}") for d in range(DC)]
        m1 = [po.tile([P, LC], BF, name=f"m1{tag}{d}") for d in range(DC)]
        m2 = [po.tile([P, LC], BF, name=f"m2{tag}{d}") for d in range(DC)]
        for d in range(DC):
            pc = ps.tile([P, LC], F32, tag="a", bufs=4)
            for qi, (qo, qs) in enumerate(QCH):
                nc.tensor.matmul(pc, Qn[qi][:qs, d * P:(d + 1) * P], P_T[qi][:qs],
                                 start=(qi == 0), stop=(qi == 1))
            nc.vector.tensor_copy(c2q_T[d], pc)
            nc.vector.tensor_tensor(m1[d], c2q_T[d], S_T[d], op=MUL)
            pq2 = ps.tile([P, LC], F32, tag="a", bufs=4)
            for qi, (qo, qs) in enumerate(QCH):
                nc.tensor.matmul(pq2, tmp[qi][:qs, d * P:(d + 1) * P], P_T[qi][:qs],
                                 start=(qi == 0), stop=(qi == 1))
            nc.vector.tensor_tensor(m2[d], pq2, S_T[d], op=MUL)
        xblocks = S_T + c2q_T + m1 + m2
        for mc in range(DC):
            pco = ps.tile([P, LC], F32, tag="a", bufs=4)
            nc.scalar.copy(pco, Spart[mc])
            for k in range(DC, 4 * DC):
                nc.tensor.matmul(pco, cqa_WT[k][:, mc * P:(mc + 1) * P],
                                 xblocks[k], start=False, stop=(k == 4 * DC - 1),
                                 skip_group_check=True)
            ob = po.tile([P, LC], BF, name=f"ob{tag}{mc}", tag="attb", bufs=2)
            nc.scalar.activation(ob, pco, AF.Identity,
                                 bias=vecs[:, mc, 3:4], scale=1.0)
            nc.sync.dma_start(out=out_t[(row0 + mc) * P:(row0 + mc + 1) * P, :],
                              in_=ob)
            nc.vector.tensor_copy(att_T[row0 + mc], ob)

    def proj_early(wt3, rhs_tiles, nk, nm):
        outs = [persist.tile([P, LK], BF, name=f"pe_{nm}{m}") for m in range(4)]
        pss = [ps.tile([P, LK], F32, name=f"pe_ps{nm}{m}", tag="b", bufs=4)
               for m in range(4)]
        for k in range(nk):
            for m in range(4):
                nc.tensor.matmul(pss[m], wt3[:, k, m * P:(m + 1) * P], rhs_tiles[k],
                                 start=(k == 0), stop=(k == nk - 1))
        for m in range(4):
            nc.vector.tensor_copy(outs[m], pss[m])
        return outs

    def proj_v_early(wt3, kv_T, nkv, nm):
        v_aug = [persist.tile([P, H, DV + 1], BF, name=f"pe_va{nm}{c}")
                 for c in range(CC)]
        pvs = [ps.tile([P, H * DV], F32, name=f"pe_pv{nm}{m}", tag="b", bufs=4)
               for m in range(4)]
        for k in range(nkv):
            for c in range(CC):
                nc.tensor.matmul(pvs[c], kv_T[k][:, c * P:(c + 1) * P], wt3[:, k, :],
                                 start=(k == 0), stop=(k == nkv - 1))
        for c in range(CC):
            nc.vector.tensor_copy(v_aug[c][:, :, 0:DV],
                                  pvs[c].rearrange("p (h d) -> p h d", h=H))
            nc.gpsimd.memset(v_aug[c][:, :, DV:DV + 1], 1.0)
        return v_aug

    stageA('q'); stageA('e')
    Spart = [s2q.tile([P, LC], BF, name=f"Spart{mc}") for mc in range(DC)]
    for mc in range(DC):
        psp = ps.tile([P, LC], F32, tag="a", bufs=4)
        for k in range(DC):
            nc.tensor.matmul(psp, cqa_WT[k][:, mc * P:(mc + 1) * P], S_T[k],
                             start=(k == 0), stop=(k == DC - 1))
        nc.scalar.copy(Spart[mc], psp)

    stageB('q'); stageB('e')
    pre_q0 = proj_early(wls[0]['sWq'], ke_T, DC, "q0")
    pre_k0 = proj_early(wls[0]['sWk'], ke_T, DC, "k0")
    pre_v0 = proj_v_early(wls[0]['sWv'], ke_T, DC, "v0")
    stageC('q', 0); stageC('e', DC)
    pools['e'].release(); pools['q'].release()
    cqaw.release(); s2q.release()

    # ---------------- phase 2: knowledge attention stack ----------------
    mp = ctx.enter_context(tc.tile_pool(name="mp", bufs=1))
    wl1pool = tc.alloc_tile_pool(name="wl1", bufs=1)
    wls[1] = alloc_wl(1, wl1pool, ('sWq', 'sWk', 'sWv', 'sWfc', 'cWq', 'cWfc'))
    wc0 = alloc_wl(0, wl1pool, ('cWq', 'cWfc'))

    def proj(wt3, rhs_tiles, nk, out_name, tagbase):
        outs = [mp.tile([P, LK], BF, name=f"{out_name}{m}", tag=f"{tagbase}{m}",
                        bufs=1) for m in range(4)]
        pss = [ps.tile([P, LK], F32, name=f"pss{m}", tag="a", bufs=4) for m in range(4)]
        for k in range(nk):
            for m in range(4):
                nc.tensor.matmul(pss[m], wt3[:, k, m * P:(m + 1) * P], rhs_tiles[k],
                                 start=(k == 0), stop=(k == nk - 1))
        for m in range(4):
            nc.vector.tensor_copy(outs[m], pss[m])
        return outs

    def proj_stream(w_dram, rhs_tiles, nk, out_name, tagbase, wpool):
        outs = [mp.tile([P, LK], BF, name=f"{out_name}{m}", tag=f"{tagbase}{m}",
                        bufs=2) for m in range(4)]
        pss = [ps.tile([P, LK], F32, name=f"pss{m}", tag="a", bufs=4) for m in range(4)]
        GS = 3
        for k0 in range(0, nk, GS):
            wt3 = wpool.tile([P, GS, H * DK], BF, name=f"w{out_name}{k0}",
                             tag="wst", bufs=2)
            nc.sync.dma_start(out=wt3, in_=w_dram[:, k0 * H * DK:(k0 + GS) * H * DK])
            for k in range(GS):
                for m in range(4):
                    nc.tensor.matmul(pss[m], wt3[:, k, m * P:(m + 1) * P],
                                     rhs_tiles[k0 + k],
                                     start=(k0 + k == 0), stop=(k0 + k == nk - 1))
        for m in range(4):
            nc.vector.tensor_copy(outs[m], pss[m])
        return outs

    def proj_v_stream(w_dram, kv_T, nkv, tag, wpool):
        v_aug = [mp.tile([P, H, DV + 1], BF, name=f"va{tag}{c}", tag=f"va{tag}{c}",
                         bufs=1) for c in range(CC)]
        pvs = [ps.tile([P, H * DV], F32, name=f"pvs{m}", tag="a", bufs=4) for m in range(4)]
        GS = 3
        for k0 in range(0, nkv, GS):
            wt3 = wpool.tile([P, GS, H * DV], BF, name=f"wv{tag}{k0}",
                             tag="wst", bufs=2)
            nc.sync.dma_start(out=wt3, in_=w_dram[:, k0 * H * DV:(k0 + GS) * H * DV])
            for k in range(GS):
                for c in range(CC):
                    nc.tensor.matmul(pvs[c], kv_T[k0 + k][:, c * P:(c + 1) * P],
                                     wt3[:, k, :],
                                     start=(k0 + k == 0), stop=(k0 + k == nkv - 1))
        for c in range(CC):
            nc.vector.tensor_copy(v_aug[c][:, :, 0:DV],
                                  pvs[c].rearrange("p (h d) -> p h d", h=H))
            nc.gpsimd.memset(v_aug[c][:, :, DV:DV + 1], 1.0)
        return v_aug

    def proj_v(wt3, kv_T, nkv, tag):
        v_aug = [mp.tile([P, H, DV + 1], BF, name=f"va{tag}{c}", tag=f"va{tag}{c}",
                         bufs=1) for c in range(CC)]
        pvs = [ps.tile([P, H * DV], F32, name=f"pvs{m}", tag="a", bufs=4) for m in range(4)]
        for k in range(nkv):
            for c in range(CC):
                nc.tensor.matmul(pvs[c], kv_T[k][:, c * P:(c + 1) * P], wt3[:, k, :],
                                 start=(k == 0), stop=(k == nkv - 1))
        for c in range(CC):
            nc.vector.tensor_copy(v_aug[c][:, :, 0:DV],
                                  pvs[c].rearrange("p (h d) -> p h d", h=H))
            nc.gpsimd.memset(v_aug[c][:, :, DV:DV + 1], 1.0)
        return v_aug

    def mha_ln(x_T, w, pre, g_ap, b_ap, tag, out_f32=False, pre_q=None,
               skip_ln=False):
        """x_T: 6 [P,LK] bf16 query-side tiles. w: (wq3, wfc3). pre: (k_T, v_aug).
        returns new 6 [P,LK] tiles = LN(fc(attn) + x_T)."""
        wq3, wfc3 = w
        k_T, v_aug = pre
        q_T = pre_q if pre_q is not None else proj(wq3, x_T, DC, f"q{tag}", "qT")
        out_T = [mp.tile([P, LK], BF, name=f"o{tag}{m}", tag=f"oT{m}", bufs=1)
                 for m in range(4)]
        for g in range(2):
            povs = []
            for hh in range(4):
                h = g * 4 + hh
                t, o = h // 2, (h % 2) * DK
                e_sb = []
                for c in range(CC):
                    pa = ps.tile([P, LK], F32, tag="a", bufs=4)
                    nc.tensor.matmul(pa, k_T[t][o:o + DK, c * P:(c + 1) * P],
                                     q_T[t][o:o + DK, :], start=True, stop=True)
                    es = mp.tile([P, LK], BF, name=f"es{tag}{h}{c}", tag="es", bufs=8)
                    nc.scalar.activation(es, pa, AF.Exp, scale=SCALE)
                    e_sb.append(es)
                pov = ps.tile([DV + 1, LK], F32, tag="b", bufs=4)
                for c in range(CC):
                    nc.tensor.matmul(pov, v_aug[c][:, h, :], e_sb[c],
                                     start=(c == 0), stop=(c == CC - 1))
                povs.append(pov)
            for hh in range(4):
                h = g * 4 + hh
                t, o = h // 2, (h % 2) * DK
                rrs = mp.tile([1, LK], F32, name=f"rrs{tag}{h}", tag="rrs", bufs=2)
                nc.vector.tensor_copy(rrs, povs[hh][DV:DV + 1, :])
                rr = mp.tile([1, LK], F32, name=f"rr{tag}{h}", tag="rr", bufs=2)
                nc.vector.reciprocal_approx_fast(out=rr, in_=rrs)
                pbc = mp.tile([DV, LK], F32, name=f"pbc{tag}{h}", tag="pbc", bufs=2)
                nc.gpsimd.partition_broadcast(pbc, rr)
                nc.vector.tensor_tensor(out_T[t][o:o + DK, :], povs[hh][:DV, :],
                                        pbc, op=MUL)
        # --- fc + residual + LN ---
        x1 = [mp.tile([P, LK], BF, name=f"x1{tag}{d}", tag=f"x1{d}", bufs=1)
              for d in range(DC)]
        for d in range(DC):
            pf = ps.tile([P, LK], F32, tag="a", bufs=4)
            for k in range(4):
                nc.tensor.matmul(pf, wfc3[:, k, d * P:(d + 1) * P], out_T[k],
                                 start=(k == 0), stop=(k == 3))
            nc.vector.tensor_tensor(x1[d], pf, x_T[d], op=ADD)
        if skip_ln:
            return x1
        ps_s = ps.tile([1, LK], F32, tag="b", bufs=4)
        ps_q = ps.tile([1, LK], F32, tag="b", bufs=4)
        sqs = [mp.tile([P, LK], BF, name=f"sq{tag}{d}", tag="sq", bufs=3)
               for d in range(DC)]
        for d in range(DC):
            nc.vector.tensor_tensor(sqs[d], x1[d], x1[d], op=MUL)
        for d in range(DC):
            nc.tensor.matmul(ps_s, ones_col, x1[d], start=(d == 0), stop=(d == DC - 1))
        for d in range(DC):
            nc.tensor.matmul(ps_q, ones_col, sqs[d], start=(d == 0), stop=(d == DC - 1))
        mu = mp.tile([1, LK], F32, name=f"mu{tag}", tag="mu", bufs=1)
        nc.scalar.activation(mu, ps_s, AF.Copy, bias=0.0, scale=1.0 / D)
        msq = mp.tile([1, LK], F32, name=f"msq{tag}", tag="msq", bufs=1)
        nc.scalar.activation(msq, ps_q, AF.Copy, bias=0.0, scale=1.0 / D)
        var = mp.tile([1, LK], F32, name=f"var{tag}", tag="var", bufs=1)
        nc.vector.tensor_tensor(var, mu, mu, op=MUL)
        nc.vector.tensor_tensor(var, msq, var, op=SUB)
        lv = mp.tile([1, LK], F32, name=f"lv{tag}", tag="lv", bufs=1)
        nc.scalar.activation(lv, var, AF.Ln, bias=eps_t, scale=1.0)
        rstd = mp.tile([1, LK], F32, name=f"rstd{tag}", tag="rstd", bufs=1)
        nc.scalar.activation(rstd, lv, AF.Exp, bias=0.0, scale=-0.5)
        c2 = mp.tile([1, LK], F32, name=f"c2{tag}", tag="c2", bufs=1)
        nc.vector.tensor_tensor(c2, mu, rstd, op=MUL)
        rstdb = mp.tile([1, LK], BF, name=f"rstdb{tag}", tag="rstdb", bufs=1)
        nc.vector.tensor_copy(rstdb, rstd)
        c2b = mp.tile([1, LK], BF, name=f"c2b{tag}", tag="c2b", bufs=1)
        nc.vector.tensor_copy(c2b, c2)
        pA = mp.tile([P, LK], BF, name=f"pA{tag}", tag="pA", bufs=1)
        nc.gpsimd.partition_broadcast(pA, rstdb)
        pC = mp.tile([P, LK], BF, name=f"pC{tag}", tag="pC", bufs=1)
        nc.gpsimd.partition_broadcast(pC, c2b)
        y = [mp.tile([P, LK], BF, name=f"y{tag}{d}", tag=f"y{tag[0]}{d}", bufs=1)
             for d in range(DC)]
        yt = [mp.tile([P, LK], BF, name=f"yt{tag}{d}", tag="yt", bufs=3)
              for d in range(DC)]
        for d in range(DC):
            nc.vector.tensor_tensor(yt[d], x1[d], pA, op=MUL)
            nc.vector.tensor_tensor(yt[d], yt[d], pC, op=SUB)
            if out_f32 and d % 2 == 0:
                nc.vector.tensor_scalar(y[d], yt[d], g_ap[d], b_ap[d],
                                        op0=MUL, op1=ADD)
            else:
                nc.scalar.activation(y[d], yt[d], AF.Identity,
                                     bias=b_ap[d], scale=g_ap[d])

        return y

    cur = ke_T
    pre_c = None
    for l in range(NL):
        w = dict(wls[l])
        if l == 0:
            w.update(wc0)
        g1 = [lnv[l][:, d, 0:1] for d in range(DC)]
        b1 = [lnv[l][:, d, 1:2] for d in range(DC)]
        g2 = [lnv[l][:, d, 2:3] for d in range(DC)]
        b2 = [lnv[l][:, d, 3:4] for d in range(DC)]
        if l == 0:
            # hoisted: cross-attn k/v depend only on att (ready after phase 1)
            wstr = tc.alloc_tile_pool(name=f"wstr{l}", bufs=1)
            pre_c = (proj_stream(dt[f'cWk{l}'], att_T, 2 * DC, f"kc{l}", "kTc", wstr),
                     proj_v_stream(dt[f'cWv{l}'], att_T, 2 * DC, f"c{l}", wstr))
            wstr.release()
            dma_wl(1, wls[1])
            dma_wl(0, wc0)
        if l == 0:
            pre_s, pq = (pre_k0, pre_v0), pre_q0
        else:
            pre_s = (proj(w['sWk'], cur, DC, f"ks{l}", "kTs"),
                     proj_v(w['sWv'], cur, DC, f"s{l}"))
            pq = None
        so = mha_ln(cur, (w['sWq'], w['sWfc']), pre_s, g1, b1, f"s{l}", pre_q=pq)
        if l + 1 < NL:
            # split the next layer's cross-kv hoist: K fills the s->c LN
            # boundary, V fills the c->s boundary
            wstrk = tc.alloc_tile_pool(name=f"wstrk{l+1}", bufs=1)
            k_next = proj_stream(dt[f'cWk{l+1}'], att_T, 2 * DC, f"kc{l+1}", "kTc", wstrk)
            wstrk.release()
        cur = mha_ln(so, (w['cWq'], w['cWfc']), pre_c, g2, b2, f"c{l}",
                     out_f32=(l == NL - 1), skip_ln=(l == NL - 1))
        if l + 1 < NL:
            wstrv = tc.alloc_tile_pool(name=f"wstrv{l+1}", bufs=1)
            v_next = proj_v_stream(dt[f'cWv{l+1}'], att_T, 2 * DC, f"c{l+1}", wstrv)
            wstrv.release()
            pre_c = (k_next, v_next)
    for d in range(DC):
        nc.sync.dma_start(out=out_t[(2 * DC + d) * P:(2 * DC + d + 1) * P, :],
                          in_=cur[d])
    wl1pool.release()
    ctx.close()


def _t128(a):
    # [n*128, w] -> [128, n*w] so each partition's DMA line is contiguous
    n = a.shape[0] // P
    return np.ascontiguousarray(
        a.reshape(n, P, a.shape[1]).transpose(1, 0, 2).reshape(P, -1))


def _t128pad(a):
    # ragged rows (LQ=160): pad to 2*128 rows then tile
    out = np.zeros((2 * P, a.shape[1]), a.dtype)
    out[:a.shape[0]] = a
    return _t128(out)


def kernel(**inputs):
    if 'nc' not in _CACHE:
        _CACHE['nc'] = _build()
    nc = _CACHE['nc']
    f = lambda x: np.ascontiguousarray(np.asarray(x), dtype=np.float32)
    bf = lambda x: np.asarray(x, dtype=np.float32).astype(NPBF)
    bfT = lambda x: np.asarray(x, dtype=np.float32).T.astype(NPBF)
    seq = f(inputs['sequences']); qry = f(inputs['query']); evd = f(inputs['evidence'])
    ke = f(inputs['knowledge_embed'])
    vecs = _t128(np.ascontiguousarray(np.stack(
        [f(inputs['w4C'])[:, 0], f(inputs['w4Q'])[:, 0],
         f(inputs['w4mlu'])[0, 0, :], f(inputs['cqa_b'])], axis=1)))
    cqa_WT = _t128(bfT(inputs['cqa_W']))
    lwb = {}
    for l in range(NL):
        lwb[f'sWq{l}'] = _t128(bf(inputs['L_sWq'][l]))
        lwb[f'sWk{l}'] = _t128(bf(inputs['L_sWk'][l]))
        lwb[f'sWv{l}'] = _t128(bf(inputs['L_sWv'][l]))
        lwb[f'sWfc{l}'] = _t128(bf(inputs['L_sWfc'][l]))
        lwb[f'cWq{l}'] = _t128(bf(inputs['L_cWq'][l]))
        lwb[f'cWk{l}'] = _t128(bf(inputs['L_cWk'][l]))
        lwb[f'cWv{l}'] = _t128(bf(inputs['L_cWv'][l]))
        lwb[f'cWfc{l}'] = _t128(bf(inputs['L_cWfc'][l]))
        lwb[f'ln{l}'] = _t128(np.ascontiguousarray(np.stack(
            [f(inputs['L_n1g'][l]), f(inputs['L_n1b'][l]),
             f(inputs['L_n2g'][l]), f(inputs['L_n2b'][l])], axis=1)))
    in_maps = []
    for b in range(B):
        m = {
            'S_nat': _t128(bf(seq[b])), 'S_T': _t128(bfT(seq[b])),
            'Q_nat': _t128pad(bf(qry[b])), 'Q_T': _t128(bfT(qry[b])),
            'E_nat': _t128pad(bf(evd[b])), 'E_T': _t128(bfT(evd[b])),
            'KE_T': _t128(bfT(ke[b])),
            'vecs': vecs, 'cqa_WT': cqa_WT,
        }
        m.update(lwb)
        in_maps.append(m)
    _CACHE['last_in_maps'] = in_maps
    res = run_bass_kernel_spmd(nc, in_maps, core_ids=list(range(B)))
    _CACHE['last_results'] = res
    outs = np.stack([np.asarray(r['out_t'], dtype=np.float32)
                     for r in res.results])                      # [B, 2304, 512]
    full = outs.transpose(0, 2, 1)                               # [B, 512, 2304]
    x1 = full[:, :, 2 * D:]                                      # pre-LN ke [B,512,768]
    muh = x1.mean(-1, keepdims=True)
    varh = x1.var(-1, keepdims=True)
    g = np.asarray(inputs['L_n2g'][NL - 1], dtype=np.float32)
    bb = np.asarray(inputs['L_n2b'][NL - 1], dtype=np.float32)
    ke_out = (x1 - muh) / np.sqrt(varh + EPS) * g + bb
    out = np.concatenate([seq, full[:, :, :2 * D], ke_out], axis=-1)
    return out

